# revision 1
# baseline (speedup 1.0000x reference)
"""Trainium2 Bass kernel for AbsoluteSinusoidal2DPE logits.

Math (flattened, N = H*W = 1024, D = 512):
    logits[b] = q[b] @ e^T + e @ (k[b] + e)^T          # [N, N] per batch

Sharding: batch dim (16) data-parallel over 8 cores, 2 batches/core; the
[N, D] embed table is replicated. Host-side prep transposes operands to
[D, N] (contraction dim on partitions) and rounds matmul operands to the
PE's fp32r format (fp32 with 11 explicit mantissa bits, single-pass
matmul at 1 column/cycle vs 4 for full fp32).

Per core: 256 matmuls (K=128, M=128, N=512). Measured steady state is at
the PE streaming floor (~34 us/batch = 128 MMs x 512 cols at the chip's
sustained ~2.0 GHz). Startup is a quadrant schedule (PSUM banks split 4/4
between m-halves, pass order A1 B1 C1 A2 D1 B2 C2 D2) whose operand
consumption order exactly matches the DMA arrival order, with a PE
pre-warm so the HAM clock gate is at full rate when real matmuls start.
Batch 0's output is staged fully in SBUF and flushed as one 4MB DMA so
batch 1's input loads own the DMA bandwidth (no mid-kernel contention);
batch 1 trickles outputs group-serially and splits the last stores
across both HWDGE rings. Relative error vs the fp32 reference ~1e-4
(absmax/scale ~4.5e-5).
"""

import numpy as np

B, H, W, D = 16, 32, 32, 512
N = H * W            # 1024
NCORES = 8
BPC = B // NCORES    # batches per core
P = 128              # partitions
KO = D // P          # 4 contraction chunks
NT = N // P          # 8 output row tiles
MH = N // 512        # 2 output column halves (PSUM bank = 512 fp32)

_PROG = None  # cached (nc) bass program, reused across kernel() calls


def _round_fp32r(x: np.ndarray) -> np.ndarray:
    """Round fp32 -> fp32r (RNE to 11 explicit mantissa bits, low 12 bits 0).

    Matches TRN2 hardware rounding (verified against DVE f32->f32r copy).
    """
    xi = x.view(np.uint32).astype(np.uint64)
    add = ((xi >> 12) & 1) + 0x7FF
    xi = (xi + add) & 0xFFFFF000
    return np.ascontiguousarray(xi.astype(np.uint32).view(np.float32))


def _build_program(n_batches: int = BPC, loop_reps: int = 0,
                   startup: str = "quadrant", prewarm: bool = True):
    """n_batches > BPC repeats the batch loop (cycling the same DRAM data);
    loop_reps > 0 wraps the whole body in a For_i hardware loop (timing
    instrument); startup="simple" disables the quadrant/pre-warm startup
    (A/B baseline). The real kernel uses the defaults."""
    import contextlib
    import concourse.mybir as mybir
    import concourse.tile as tile
    from concourse import bacc

    F32 = mybir.dt.float32
    F32R = mybir.dt.float32r

    nc = bacc.Bacc()
    qt_d = nc.dram_tensor("qt", [BPC, D, N], F32R, kind="ExternalInput")
    kt_d = nc.dram_tensor("kt", [BPC, D, N], F32, kind="ExternalInput")
    et_d = nc.dram_tensor("et", [D, N], F32R, kind="ExternalInput")
    out_d = nc.dram_tensor("out", [BPC, N, N], F32, kind="ExternalOutput")

    with tile.TileContext(nc) as tc:
        with (
            tc.tile_pool(name="etp", bufs=1) as etp,
            tc.tile_pool(name="inp", bufs=2) as inp,
            tc.tile_pool(name="outp", bufs=8) as outp,
            tc.tile_pool(name="stg", bufs=1) as stg,
            tc.tile_pool(name="ps", bufs=1, space="PSUM") as psp,
        ):
          loop_cm = tc.For_i(0, loop_reps, 1) if loop_reps else contextlib.nullcontext()
          with loop_cm:
            # embed^T resident: [128, KO, N]
            et = etp.tile([P, KO, N], F32R, name="et")
            et_src = et_d.rearrange("(ko p) m -> p ko m", p=P)

            if startup == "quadrant" and prewarm:
                # PE pre-warm: dummy matmuls on a zeroed scratch tile while
                # the first input DMAs are in flight, so the HAM clock gate
                # reaches full rate before real matmuls start
                warm = etp.tile([P, 128], F32R, name="warm")
                nc.vector.memset(warm[:].bitcast(F32), 0.0)
                warm_ps = psp.tile([P, 512], F32, tag="ps7", name="warm_ps")
                for _ in range(16):
                    nc.tensor.matmul(warm_ps[:, 0:128], warm[:], warm[:],
                                     start=True, stop=True)

            for b in range(n_batches):
                bi = b % BPC
                qt = inp.tile([P, KO, N], F32R, tag="qt")
                kt = inp.tile([P, KO, N], F32, tag="kt")
                kpe = inp.tile([P, KO, N], F32R, tag="kpe")
                qt_src = qt_d[bi].rearrange("(ko p) m -> p ko m", p=P)
                kt_src = kt_d[bi].rearrange("(ko p) m -> p ko m", p=P)
                if b == 0 and startup == "quadrant":
                    # DMA arrival order matched to the quadrant schedule's
                    # consumption order (A1: t1/mh0/nt0-3 -> B1: t1/mh1 ->
                    # C1: t2/mh0 -> D1: t2/mh1 -> half nt4-7). kpe adds are
                    # split per m-half so term2-mh0 only waits on kt[0:512].
                    for ko in range(KO):
                        nc.sync.dma_start(qt[:, ko, 0:512], qt_src[:, ko, 0:512])
                        nc.sync.dma_start(et[:, ko, 0:512], et_src[:, ko, 0:512])
                    for ko in range(KO):
                        nc.sync.dma_start(et[:, ko, 512:N], et_src[:, ko, 512:N])
                    for ko in range(KO):
                        nc.sync.dma_start(kt[:, ko, 0:512], kt_src[:, ko, 0:512])
                        nc.vector.tensor_add(
                            kpe[:, ko, 0:512], kt[:, ko, 0:512],
                            et[:, ko, 0:512].bitcast(F32))
                    for ko in range(KO):
                        nc.sync.dma_start(qt[:, ko, 512:N], qt_src[:, ko, 512:N])
                    for ko in range(KO):
                        nc.sync.dma_start(kt[:, ko, 512:N], kt_src[:, ko, 512:N])
                        nc.vector.tensor_add(
                            kpe[:, ko, 512:N], kt[:, ko, 512:N],
                            et[:, ko, 512:N].bitcast(F32))
                else:
                    if b == 0:  # simple-startup baseline: load et here
                        for ko in range(KO):
                            nc.sync.dma_start(et[:, ko], et_src[:, ko])
                    # qt ahead of kt: the batch's term1 (first 32 matmuls)
                    # needs all qt chunks; kpe (term2) is consumed ~7us later
                    for ko in range(KO):
                        nc.sync.dma_start(qt[:, ko], qt_src[:, ko])
                    for ko in range(KO):
                        nc.sync.dma_start(kt[:, ko], kt_src[:, ko])
                        # (k + e) rounded to fp32r via DVE output dtype
                        nc.vector.tensor_add(
                            kpe[:, ko], kt[:, ko], et[:, ko].bitcast(F32)
                        )

                out_rows = out_d[bi].rearrange("(nt p) m -> nt p m", p=P)

                def mm_t1(ps, nt, ko, ms, start):
                    nc.tensor.matmul(
                        ps[:], qt[:, ko, nt * P:(nt + 1) * P], et[:, ko, ms],
                        start=start, stop=False)

                def mm_t2(ps, nt, ko, ms, stop):
                    nc.tensor.matmul(
                        ps[:], et[:, ko, nt * P:(nt + 1) * P], kpe[:, ko, ms],
                        start=False, stop=stop)

                if b == 0 and startup == "quadrant":
                    # startup quadrant schedule, sequenced so each pass's
                    # operands arrive exactly in DMA order:
                    #   A1(t1,mh0,lo) B1(t1,mh1,lo) C1(t2,mh0,lo+close)
                    #   A2(t1,mh0,hi) D1(t2,mh1,lo+close) B2(t1,mh1,hi)
                    #   C2(t2,mh0,hi+close) D2(t2,mh1,hi+close)
                    # mh0 quadrants use banks ps0-3, mh1 quadrants ps4-7
                    lo, hi = list(range(4)), list(range(4, 8))
                    ps_q = {}

                    def open_t1(mh, nts):
                        ms = slice(mh * 512, (mh + 1) * 512)
                        for j, nt in enumerate(nts):
                            tag = f"ps{mh * 4 + j}"
                            ps_q[(mh, nt)] = psp.tile(
                                [P, 512], F32, tag=tag, name=tag)
                        for ko in range(KO):
                            for nt in nts:
                                mm_t1(ps_q[(mh, nt)], nt, ko, ms, ko == 0)

                    # batch-0 output is staged fully in SBUF and flushed as
                    # one 4MB DMA whose dependency (the last close copy)
                    # fires only after batch-1's input loads have the DMA
                    # bandwidth to themselves -- avoids mid-kernel contention
                    ob0 = stg.tile([P, NT, N], F32, name="ob0")

                    def close_t2(mh, nts):
                        ms = slice(mh * 512, (mh + 1) * 512)
                        for ko in range(KO):
                            for nt in nts:
                                mm_t2(ps_q[(mh, nt)], nt, ko, ms, ko == KO - 1)
                        for nt in nts:
                            nc.vector.tensor_copy(ob0[:, nt, ms], ps_q[(mh, nt)][:])

                    open_t1(0, lo)    # A1: qt-h0 + et-m0
                    open_t1(1, lo)    # B1: + et-m1
                    close_t2(0, lo)   # C1: + kpe-m0 (kt-h0)
                    open_t1(0, hi)    # A2: + qt-h1
                    close_t2(1, lo)   # D1: + kpe-m1 (kt-h1)
                    open_t1(1, hi)    # B2
                    close_t2(0, hi)   # C2
                    close_t2(1, hi)   # D2
                    nc.scalar.dma_start(
                        out_d[bi].rearrange("(nt p) m -> p nt m", p=P), ob0[:])
                else:
                    # steady/tail: group-serial so outputs trickle out
                    for mh in range(MH):
                        ms = slice(mh * 512, (mh + 1) * 512)
                        last_pass = (b == n_batches - 1) and (mh == MH - 1)
                        for nt in range(NT):
                            ps = psp.tile([P, 512], F32, tag=f"ps{nt}",
                                          name=f"ps{nt}")
                            for ko in range(KO):
                                mm_t1(ps, nt, ko, ms, ko == 0)
                            for ko in range(KO):
                                mm_t2(ps, nt, ko, ms, ko == KO - 1)
                            ob = outp.tile([P, 512], F32, tag="ob")
                            if last_pass and nt >= NT - 2:
                                # tail: split the final stores across both
                                # HWDGE rings so the last write's HBM receipt
                                # overlaps the other half's stream
                                nc.vector.tensor_copy(ob[:, 0:256], ps[:, 0:256])
                                nc.vector.tensor_copy(ob[:, 256:512], ps[:, 256:512])
                                nc.scalar.dma_start(
                                    out_rows[nt][:, mh * 512:mh * 512 + 256],
                                    ob[:, 0:256])
                                nc.sync.dma_start(
                                    out_rows[nt][:, mh * 512 + 256:(mh + 1) * 512],
                                    ob[:, 256:512])
                            else:
                                nc.vector.tensor_copy(ob[:], ps[:])
                                nc.scalar.dma_start(out_rows[nt][:, ms], ob[:])

    nc.compile()
    return nc


def kernel(q: np.ndarray, k: np.ndarray, embed: np.ndarray) -> np.ndarray:
    global _PROG
    from concourse import bass_utils

    q = np.asarray(q)
    k = np.asarray(k)
    embed = np.asarray(embed)
    assert q.shape == (B, H, W, D) and k.shape == (B, H, W, D)
    assert embed.shape == (H, W, D)

    qf = q.reshape(B, N, D).astype(np.float32, copy=False)
    kf = k.reshape(B, N, D).astype(np.float32, copy=False)
    ef = embed.reshape(N, D).astype(np.float32, copy=False)

    # [B, D, N] contiguous transposes; q and e pre-rounded to fp32r
    qt = _round_fp32r(np.ascontiguousarray(qf.transpose(0, 2, 1)))
    kt = np.ascontiguousarray(kf.transpose(0, 2, 1))
    et = _round_fp32r(np.ascontiguousarray(ef.T))

    if _PROG is None:
        _PROG = _build_program()
    nc = _PROG

    in_maps = []
    for c in range(NCORES):
        sl = slice(c * BPC, (c + 1) * BPC)
        in_maps.append({"qt": qt[sl], "kt": kt[sl], "et": et})

    res = bass_utils.run_bass_kernel_spmd(nc, in_maps, core_ids=list(range(NCORES)))
    outs = [r["out"] for r in res.results]  # each [BPC, N, N]
    full = np.concatenate(outs, axis=0)     # [B, N, N]
    return np.ascontiguousarray(full.reshape(B, H, W, H, W))



# revision 3
# speedup vs baseline: 1.8871x; 1.8871x over previous
"""Trainium2 Bass kernel for AbsoluteSinusoidal2DPE logits.

Math (flattened, N = H*W = 1024, D = 512):
    logits[b] = q[b] @ e^T + e @ (k[b] + e)^T          # [N, N] per batch

Key structure: the embed table is separable, e[(i,j), :] = eh[i, :] + ew[j, :]
(make_embed builds it as an outer sum of two 1-D tables). With the rank-64
basis E2 = [eh; ew] ([64, D]) and the 0/1 selection matrix
sel[m, (a,b)] = [m == a] + [m == 32 + b] ([64, N], identical for rows and
columns), the logits factor exactly as

    ABt = E2 @ q[b]^T                      # [64, N]   (A^T; B^T stacked)
    CD  = E2 @ k[b]^T + CDe                # [64, N]   (C; D stacked)
    logits[b] = sel^T @ CD + ABt^T @ sel   # [N, N]

where CDe = E2 @ e^T is batch-independent and itself separable from the tiny
Gram matrix G = E2 @ E2^T ([64, 64], host-computed):
CDe[m, (a,b)] = G[m, a] + G[m, 32 + b].

This cuts per-batch PE work ~2.7x vs the dense formulation (24.6K vs 65.5K
PE cycles) and more importantly lets q, k ship as fp16 (logits are uniformly
large, |logits| in [325, 1115], so the 2e-2 rel-err gate is an absolute
budget of ~6.5; fp16 inputs + fp32r expansion measure ~5e-4). Per-core DMA
drops from ~18 MB to ~12.6 MB (2 MB fp16 in + 4 MB fp32 out per batch +
0.6 MB constants), which is the new floor at ~358 GB/s/core.

Sharding: batch dim (16) data-parallel over 8 cores, 2 batches/core.

Per core per batch: 16 in-projection matmuls (fp16, K=128, M=64, N=512) +
32 expansion matmuls (fp32r, K=64, M=128, N=512), 4 DVE ops to round
ABt/CD into fp32r SBUF, 16 PSUM->SBUF copies, 8 output DMAs of [128, 1024].
"""

import numpy as np

B, H, W, D = 16, 32, 32, 512
N = H * W            # 1024
NCORES = 8
BPC = B // NCORES    # batches per core
P = 128              # partitions
KO = D // P          # 4 contraction chunks
NT = N // P          # 8 output row tiles
R = 64               # separable basis rank (32 rows + 32 cols)

_PROG = None  # cached bass program, reused across kernel() calls


def _build_program(n_batches: int = BPC, loop_reps: int = 0,
                   prewarm: bool = True):
    """n_batches > BPC repeats the batch loop (cycling the same DRAM data);
    loop_reps > 0 wraps the whole body in a For_i hardware loop (timing
    instrument; prewarm is skipped there). The real kernel uses defaults."""
    import contextlib
    import concourse.mybir as mybir
    import concourse.tile as tile
    from concourse import bacc

    F32 = mybir.dt.float32
    F32R = mybir.dt.float32r
    F16 = mybir.dt.float16

    nc = bacc.Bacc()
    qt_d = nc.dram_tensor("qt", [BPC, D, N], F16, kind="ExternalInput")
    kt_d = nc.dram_tensor("kt", [BPC, D, N], F16, kind="ExternalInput")
    e2t_d = nc.dram_tensor("e2t", [D, R], F16, kind="ExternalInput")
    sel_d = nc.dram_tensor("sel", [R, N], F32R, kind="ExternalInput")
    cde_d = nc.dram_tensor("cde", [R, N], F32, kind="ExternalInput")
    out_d = nc.dram_tensor("out", [BPC, N, N], F32, kind="ExternalOutput")

    with tile.TileContext(nc) as tc:
        with (
            tc.tile_pool(name="cst", bufs=1) as cst,
            tc.tile_pool(name="inp", bufs=2) as inp,
            tc.tile_pool(name="ab", bufs=2) as abp,
            tc.tile_pool(name="outp", bufs=4) as outp,
            tc.tile_pool(name="ps", bufs=1, space="PSUM") as psp,
        ):
          loop_cm = tc.For_i(0, loop_reps, 1) if loop_reps else contextlib.nullcontext()
          with loop_cm:
            e2t = cst.tile([P, KO, R], F16, name="e2t")
            sel = cst.tile([R, N], F32R, name="sel")
            cde = cst.tile([R, N], F32, name="cde")
            e2t_src = e2t_d.rearrange("(ko p) m -> p ko m", p=P)
            for ko in range(KO):
                nc.sync.dma_start(e2t[:, ko], e2t_src[:, ko])

            if prewarm and not loop_reps:
                # PE pre-warm: dummy matmuls while the first input DMAs are
                # in flight, so the HAM clock gate is at full rate when real
                # matmuls start
                warm = cst.tile([P, 128], F32R, name="warm")
                nc.vector.memset(warm[:].bitcast(F32), 0.0)
                warm_ps = psp.tile([P, 512], F32, tag="po3", name="warm_ps")
                for _ in range(16):
                    nc.tensor.matmul(warm_ps[:, 0:128], warm[:], warm[:],
                                     start=True, stop=True)

            for b in range(n_batches):
                bi = b % BPC
                qt = inp.tile([P, KO, N], F16, tag="qt", name="qt")
                kt = inp.tile([P, KO, N], F16, tag="kt", name="kt")
                qt_src = qt_d[bi].rearrange("(ko p) m -> p ko m", p=P)
                kt_src = kt_d[bi].rearrange("(ko p) m -> p ko m", p=P)
                for ko in range(KO):
                    nc.sync.dma_start(qt[:, ko], qt_src[:, ko])
                if b == 0:
                    # sel before kt (expansion needs it right after CD is
                    # ready), cde after kt (CD add is the last input dep)
                    nc.sync.dma_start(sel[:], sel_d[:, :])
                for ko in range(KO):
                    nc.sync.dma_start(kt[:, ko], kt_src[:, ko])
                if b == 0:
                    nc.sync.dma_start(cde[:], cde_d[:, :])

                # in-projections: ABt = E2 @ q^T, CDk = E2 @ k^T  [64, N]
                pa = [psp.tile([R, 512], F32, tag=f"pa{h}", name=f"pa{h}")
                      for h in range(2)]
                pk = [psp.tile([R, 512], F32, tag=f"pk{h}", name=f"pk{h}")
                      for h in range(2)]
                for ko in range(KO):
                    for h in range(2):
                        nc.tensor.matmul(
                            pa[h][:], e2t[:, ko], qt[:, ko, h * 512:(h + 1) * 512],
                            start=(ko == 0), stop=(ko == KO - 1))
                for ko in range(KO):
                    for h in range(2):
                        nc.tensor.matmul(
                            pk[h][:], e2t[:, ko], kt[:, ko, h * 512:(h + 1) * 512],
                            start=(ko == 0), stop=(ko == KO - 1))

                # round into fp32r SBUF operands for the expansion matmuls
                abt = abp.tile([R, N], F32R, tag="abt", name="abt")
                cd = abp.tile([R, N], F32R, tag="cd", name="cd")
                for h in range(2):
                    hs = slice(h * 512, (h + 1) * 512)
                    nc.vector.tensor_copy(abt[:, hs], pa[h][:])
                    nc.vector.tensor_add(cd[:, hs], pk[h][:], cde[:, hs])

                # expansion: out rows tile nt = sel^T @ CD + ABt^T @ sel
                out_rows = out_d[bi].rearrange("(nt p) m -> nt p m", p=P)
                for nt in range(NT):
                    t0 = f"po{(nt % 2) * 2}"
                    t1 = f"po{(nt % 2) * 2 + 1}"
                    psA = psp.tile([P, 512], F32, tag=t0, name=t0)
                    psB = psp.tile([P, 512], F32, tag=t1, name=t1)
                    lhs_sel = sel[:, nt * P:(nt + 1) * P]
                    lhs_ab = abt[:, nt * P:(nt + 1) * P]
                    nc.tensor.matmul(psA[:], lhs_sel, cd[:, 0:512],
                                     start=True, stop=False)
                    nc.tensor.matmul(psB[:], lhs_sel, cd[:, 512:N],
                                     start=True, stop=False)
                    nc.tensor.matmul(psA[:], lhs_ab, sel[:, 0:512],
                                     start=False, stop=True)
                    nc.tensor.matmul(psB[:], lhs_ab, sel[:, 512:N],
                                     start=False, stop=True)
                    ob = outp.tile([P, N], F32, tag="ob", name="ob")
                    nc.vector.tensor_copy(ob[:, 0:512], psA[:])
                    nc.vector.tensor_copy(ob[:, 512:N], psB[:])
                    last = (b == n_batches - 1) and (nt == NT - 1)
                    if last:
                        # split the final store across both HWDGE rings so
                        # the last write's HBM receipt overlaps the other
                        # half's stream
                        nc.scalar.dma_start(out_rows[nt][:, 0:512], ob[:, 0:512])
                        nc.sync.dma_start(out_rows[nt][:, 512:N], ob[:, 512:N])
                    else:
                        nc.scalar.dma_start(out_rows[nt], ob[:])

    nc.compile()
    return nc


def _make_consts(embed: np.ndarray):
    """Host-side prep of the tiny batch-independent operands."""
    ef = embed.reshape(N, D).astype(np.float32)
    eh = ef[0:N:W]                      # embed[:, 0, :]   [32, D]
    ew = ef[0:W] - ef[0]                # embed[0, :, :] - embed[0, 0, :]
    e2 = np.concatenate([eh, ew], axis=0)            # [64, D]
    e2t = np.ascontiguousarray(e2.T).astype(np.float16)  # [D, 64]
    g = e2 @ e2.T                                    # [64, 64] Gram
    cde = np.ascontiguousarray(
        (g[:, :W, None] + g[:, None, W:]).reshape(R, N))  # E2 @ e^T
    sel = np.zeros((R, N), np.float32)
    idx = np.arange(N)
    sel[idx // W, idx] = 1.0
    sel[W + idx % W, idx] = 1.0
    return e2t, sel, cde


def kernel(q: np.ndarray, k: np.ndarray, embed: np.ndarray) -> np.ndarray:
    global _PROG
    from concourse import bass_utils

    q = np.asarray(q)
    k = np.asarray(k)
    embed = np.asarray(embed)
    assert q.shape == (B, H, W, D) and k.shape == (B, H, W, D)
    assert embed.shape == (H, W, D)

    qf = q.reshape(B, N, D).astype(np.float32, copy=False)
    kf = k.reshape(B, N, D).astype(np.float32, copy=False)

    # [B, D, N] fp16 transposes (RNE cast, matches device numerics)
    qt = np.ascontiguousarray(qf.transpose(0, 2, 1)).astype(np.float16)
    kt = np.ascontiguousarray(kf.transpose(0, 2, 1)).astype(np.float16)
    e2t, sel, cde = _make_consts(embed)

    if _PROG is None:
        _PROG = _build_program()
    nc = _PROG

    in_maps = []
    for c in range(NCORES):
        sl = slice(c * BPC, (c + 1) * BPC)
        in_maps.append({"qt": qt[sl], "kt": kt[sl],
                        "e2t": e2t, "sel": sel, "cde": cde})

    res = bass_utils.run_bass_kernel_spmd(nc, in_maps, core_ids=list(range(NCORES)))
    outs = [r["out"] for r in res.results]  # each [BPC, N, N]
    full = np.concatenate(outs, axis=0)     # [B, N, N]
    return np.ascontiguousarray(full.reshape(B, H, W, H, W))


# revision 9
# speedup vs baseline: 2.3497x; 1.2452x over previous
"""Trainium2 Bass kernel for AbsoluteSinusoidal2DPE logits.

Math (flattened, N = H*W = 1024, D = 512):
    logits[b] = q[b] @ e^T + e @ (k[b] + e)^T          # [N, N] per batch

Key structure: the embed table is separable, e[(i,j), :] = eh[i, :] + ew[j, :]
(make_embed builds it as an outer sum of two 1-D tables). With the rank-64
basis E2 = [eh; ew] ([64, D]) and the 0/1 selection matrix
sel[m, (a,b)] = [m == a] + [m == 32 + b] ([64, N], identical for rows and
columns), the logits factor exactly as

    ABt = E2 @ q[b]^T                      # [64, N]   (A^T; B^T stacked)
    CD  = E2 @ k[b]^T + CDe                # [64, N]   (C; D stacked)
    logits[b] = sel^T @ CD + ABt^T @ sel   # [N, N]

where CDe = E2 @ e^T is batch-independent and itself separable from the tiny
Gram matrix G = E2 @ E2^T ([64, 64], host-computed):
CDe[m, (a,b)] = G[m, a] + G[m, 32 + b].

This cuts per-batch PE work ~2.7x vs the dense formulation (24.6K vs 65.5K
PE cycles) and more importantly lets q, k ship as fp16 (logits are uniformly
large, |logits| in [325, 1115], so the 2e-2 rel-err gate is an absolute
budget of ~6.5; fp16 inputs + fp32r expansion measure ~5e-4). Per-core DMA
drops from ~18 MB to ~12.6 MB (2 MB fp16 in + 4 MB fp32 out per batch +
0.6 MB constants), which is the new floor at ~358 GB/s/core.

Sharding: batch dim (16) data-parallel over 8 cores, 2 batches/core.

Scheduling: k^T loads before q^T (CD is the longer dependency), the
expansion is emitted in 4 dependency-ordered quadrant groups (row-halves x
column-halves) so the first output stores issue as soon as the first halves
of ABt/CD close, and the input pool is single-buffered so the next batch's
loads become wire-ready after this batch's in-projections rather than
racing ahead of this batch's output stores on the shared HBM path.
"""

import numpy as np

B, H, W, D = 16, 32, 32, 512
N = H * W            # 1024
NCORES = 8
BPC = B // NCORES    # batches per core
P = 128              # partitions
KO = D // P          # 4 contraction chunks
NT = N // P          # 8 output row tiles
R = 64               # separable basis rank (32 rows + 32 cols)

_PROG = None  # cached bass program, reused across kernel() calls


def _build_program(n_batches: int = BPC, loop_reps: int = 0,
                   prewarm: bool = True, inp_bufs: int = 1,
                   sync_stores: int = 3, act_copies: bool = True):
    """n_batches > BPC repeats the batch loop (cycling the same DRAM data);
    loop_reps > 0 wraps the whole body in a For_i hardware loop (timing
    instrument; prewarm is skipped there). The real kernel uses defaults."""
    import contextlib
    import concourse.mybir as mybir
    import concourse.tile as tile
    from concourse import bacc

    F32 = mybir.dt.float32
    F32R = mybir.dt.float32r
    F16 = mybir.dt.float16

    nc = bacc.Bacc()
    qt_d = nc.dram_tensor("qt", [BPC, D, N], F16, kind="ExternalInput")
    kt_d = nc.dram_tensor("kt", [BPC, D, N], F16, kind="ExternalInput")
    e2t_d = nc.dram_tensor("e2t", [D, R], F16, kind="ExternalInput")
    sel_d = nc.dram_tensor("sel", [R, N], F32R, kind="ExternalInput")
    cde_d = nc.dram_tensor("cde", [R, N], F32, kind="ExternalInput")
    out_d = nc.dram_tensor("out", [BPC, N, N], F32, kind="ExternalOutput")

    with tile.TileContext(nc) as tc:
        with (
            tc.tile_pool(name="cst", bufs=1) as cst,
            tc.tile_pool(name="inp", bufs=inp_bufs) as inp,
            tc.tile_pool(name="ab", bufs=2) as abp,
            tc.tile_pool(name="outp", bufs=6) as outp,
            tc.tile_pool(name="ps", bufs=1, space="PSUM") as psp,
        ):
          loop_cm = tc.For_i(0, loop_reps, 1) if loop_reps else contextlib.nullcontext()
          with loop_cm:
            e2t = cst.tile([P, KO, R], F16, name="e2t")
            sel = cst.tile([R, N], F32R, name="sel")
            cde = cst.tile([R, N], F32, name="cde")
            e2t_src = e2t_d.rearrange("(ko p) m -> p ko m", p=P)
            # single descriptor: HWDGE issue slots (~650ns each) dominate
            # tiny transfers
            nc.sync.dma_start(e2t[:], e2t_src[:, :])

            if prewarm and not loop_reps:
                # PE pre-warm: dummy matmuls while the first input DMAs are
                # in flight, so the HAM clock gate is at full rate when real
                # matmuls start
                warm = cst.tile([P, 128], F32R, name="warm")
                nc.vector.memset(warm[:].bitcast(F32), 0.0)
                warm_ps = psp.tile([P, 512], F32, tag="po3", name="warm_ps")
                for _ in range(16):
                    nc.tensor.matmul(warm_ps[:, 0:128], warm[:], warm[:],
                                     start=True, stop=True)

            for b in range(n_batches):
                bi = b % BPC
                qt = inp.tile([P, KO, N], F16, tag="qt", name="qt")
                kt = inp.tile([P, KO, N], F16, tag="kt", name="kt")
                qt_src = qt_d[bi].rearrange("(ko p) m -> p ko m", p=P)
                kt_src = kt_d[bi].rearrange("(ko p) m -> p ko m", p=P)
                # kt before qt: CD (k-side) is the longer dependency chain
                # (needs cde too); q-side ABt closes the expansion groups.
                if b == 0:
                    nc.sync.dma_start(cde[:], cde_d[:, :])
                for ko in range(KO):
                    nc.sync.dma_start(kt[:, ko], kt_src[:, ko])
                if b == 0:
                    nc.sync.dma_start(sel[:], sel_d[:, :])
                for ko in range(KO):
                    nc.sync.dma_start(qt[:, ko], qt_src[:, ko])

                # in-projections: CDk = E2 @ k^T, ABt = E2 @ q^T  [64, N],
                # k-side first (DMA arrival order)
                pk = [psp.tile([R, 512], F32, tag=f"pk{h}", name=f"pk{h}")
                      for h in range(2)]
                pa = [psp.tile([R, 512], F32, tag=f"pa{h}", name=f"pa{h}")
                      for h in range(2)]
                abt = abp.tile([R, N], F32R, tag="abt", name="abt")
                cd = abp.tile([R, N], F32R, tag="cd", name="cd")
                for ko in range(KO):
                    for h in range(2):
                        nc.tensor.matmul(pk[h][:], e2t[:, ko],
                                         kt[:, ko, h * 512:(h + 1) * 512],
                                         start=(ko == 0), stop=(ko == KO - 1))
                for h in range(2):
                    hs = slice(h * 512, (h + 1) * 512)
                    nc.vector.tensor_add(cd[:, hs], pk[h][:], cde[:, hs])
                for ko in range(KO):
                    for h in range(2):
                        nc.tensor.matmul(pa[h][:], e2t[:, ko],
                                         qt[:, ko, h * 512:(h + 1) * 512],
                                         start=(ko == 0), stop=(ko == KO - 1))
                for h in range(2):
                    hs = slice(h * 512, (h + 1) * 512)
                    nc.vector.tensor_copy(abt[:, hs], pa[h][:])

                # expansion: out rows tile nt = sel^T @ CD + ABt^T @ sel.
                # The first `sync_stores` row tiles store via the sync ring:
                # in HWDGE order they sit between this batch's loads and the
                # next batch's, so the next batch's loads cannot jump ahead
                # of this batch's first output stores on the shared HBM path.
                out_rows = out_d[bi].rearrange("(nt p) m -> nt p m", p=P)
                for nt in range(NT):
                    t0 = f"po{(nt % 2) * 2}"
                    t1 = f"po{(nt % 2) * 2 + 1}"
                    psA = psp.tile([P, 512], F32, tag=t0, name=t0)
                    psB = psp.tile([P, 512], F32, tag=t1, name=t1)
                    lhs_sel = sel[:, nt * P:(nt + 1) * P]
                    lhs_ab = abt[:, nt * P:(nt + 1) * P]
                    nc.tensor.matmul(psA[:], lhs_sel, cd[:, 0:512],
                                     start=True, stop=False)
                    nc.tensor.matmul(psB[:], lhs_sel, cd[:, 512:N],
                                     start=True, stop=False)
                    nc.tensor.matmul(psA[:], lhs_ab, sel[:, 0:512],
                                     start=False, stop=True)
                    nc.tensor.matmul(psB[:], lhs_ab, sel[:, 512:N],
                                     start=False, stop=True)
                    ob = outp.tile([P, N], F32, tag="ob", name="ob")
                    if act_copies and nt in (2, 5):
                        # offload a couple of whole row tiles to the scalar
                        # (Activation) engine to keep DVE off the critical
                        # path; ACT is slower per copy so whole-tile grants
                        # beat half-tile splits (store waits on both halves)
                        nc.scalar.copy(ob[:, 0:512], psA[:])
                        nc.scalar.copy(ob[:, 512:N], psB[:])
                    else:
                        nc.vector.tensor_copy(ob[:, 0:512], psA[:])
                        nc.vector.tensor_copy(ob[:, 512:N], psB[:])
                    last = (b == n_batches - 1) and (nt == NT - 1)
                    if last:
                        # split the final store across both HWDGE rings
                        nc.scalar.dma_start(out_rows[nt][:, 0:512], ob[:, 0:512])
                        nc.sync.dma_start(out_rows[nt][:, 512:N], ob[:, 512:N])
                    elif nt < sync_stores:
                        nc.sync.dma_start(out_rows[nt], ob[:])
                    else:
                        nc.scalar.dma_start(out_rows[nt], ob[:])

    nc.compile()
    return nc


def _make_consts(embed: np.ndarray):
    """Host-side prep of the tiny batch-independent operands."""
    ef = embed.reshape(N, D).astype(np.float32)
    eh = ef[0:N:W]                      # embed[:, 0, :]   [32, D]
    ew = ef[0:W] - ef[0]                # embed[0, :, :] - embed[0, 0, :]
    e2 = np.concatenate([eh, ew], axis=0)            # [64, D]
    e2t = np.ascontiguousarray(e2.T).astype(np.float16)  # [D, 64]
    g = e2 @ e2.T                                    # [64, 64] Gram
    cde = np.ascontiguousarray(
        (g[:, :W, None] + g[:, None, W:]).reshape(R, N))  # E2 @ e^T
    sel = np.zeros((R, N), np.float32)
    idx = np.arange(N)
    sel[idx // W, idx] = 1.0
    sel[W + idx % W, idx] = 1.0
    return e2t, sel, cde


def kernel(q: np.ndarray, k: np.ndarray, embed: np.ndarray) -> np.ndarray:
    global _PROG
    from concourse import bass_utils

    q = np.asarray(q)
    k = np.asarray(k)
    embed = np.asarray(embed)
    assert q.shape == (B, H, W, D) and k.shape == (B, H, W, D)
    assert embed.shape == (H, W, D)

    qf = q.reshape(B, N, D).astype(np.float32, copy=False)
    kf = k.reshape(B, N, D).astype(np.float32, copy=False)

    # [B, D, N] fp16 transposes (RNE cast, matches device numerics)
    qt = np.ascontiguousarray(qf.transpose(0, 2, 1)).astype(np.float16)
    kt = np.ascontiguousarray(kf.transpose(0, 2, 1)).astype(np.float16)
    e2t, sel, cde = _make_consts(embed)

    if _PROG is None:
        _PROG = _build_program()
    nc = _PROG

    in_maps = []
    for c in range(NCORES):
        sl = slice(c * BPC, (c + 1) * BPC)
        in_maps.append({"qt": qt[sl], "kt": kt[sl],
                        "e2t": e2t, "sel": sel, "cde": cde})

    res = bass_utils.run_bass_kernel_spmd(nc, in_maps, core_ids=list(range(NCORES)))
    outs = [r["out"] for r in res.results]  # each [BPC, N, N]
    full = np.concatenate(outs, axis=0)     # [B, N, N]
    return np.ascontiguousarray(full.reshape(B, H, W, H, W))
